# revision 16
# baseline (speedup 1.0000x reference)
"""Trainium2 Bass kernel for nn_DecoderLayer_70205535421363 (v2).

Decoder layer (pre-LN, T5-style RMSNorm, QK-norm attention + gated-silu MLP)
B=2, S=2048, D=2048, H=16, HD=128, F=8192, fp32 in/out.

8 cores = 2 batches x 4 query blocks of 512, single-launch, no collectives.

v2 design vs the original fused kernel:
  - Per-core COLUMN ROTATION: core (b, j) stores x[b].T with columns rotated
    left by 512*j, so its own query block is always columns 0..511 and the
    causal structure becomes uniform across cores: rotated key tiles r<=3 get
    a shared triangular mask; later tiles are either fully allowed (wrapped
    past keys) or fully excluded (future keys). Exclusion is done with DATA,
    not instructions: the host zeroes V (rvz) and K (qkwg scale) for excluded
    tiles, so excluded keys contribute exp(0)=1 to the softmax denominator,
    which the host-corrected bias (ngc) subtracts exactly.
  - Direct-KT: K^T[hd, s] is computed directly (wk chunk stationary, x
    moving), eliminating all K transposes and copies. Per-head k-rmsnorm
    scales come from N=1 ones-matmuls on a squared copy of K^T.
  - Transposed AV: att^T[hd, q] = sum_r V_chunk^T @ pr accumulated in PSUM
    (V chunk stationary), plus a broadcast-denominator matmul with an all-ones
    stationary.  No per-qs AV matmuls, no attention-output transposes.
  - rv (1/rms of x) is computed on the host and folded into V (rvz).
  - bf16 everywhere in attention except: q/k psums + norms (f32 accumulation),
    softmax logits (f32 psum), residual (f32), MLP psums (f32).
Two half-passes of 8 heads each keep SBUF under budget while streaming x
only twice.
"""
import numpy as np
import ml_dtypes
from contextlib import ExitStack

import jax
import jax.numpy as jnp
from jax.sharding import Mesh, PartitionSpec, NamedSharding
from jax.experimental.shard_map import shard_map

import concourse.bass as bass
import concourse.tile as tile
import concourse.mybir as mybir
from concourse.bass2jax import _bass_exec_p, install_neuronx_cc_hook, partition_id_tensor
from concourse.vector_clock import ScopedClock
from concourse.masks import make_identity

F32 = mybir.dt.float32
F32R = mybir.dt.float32r
BF16 = mybir.dt.bfloat16
AF = mybir.ActivationFunctionType
bf16 = ml_dtypes.bfloat16

B, S, D, H, HD, F = 2, 2048, 2048, 16, 128, 8192
EPS = 1e-6
SB = 512          # seq positions per core (queries / MLP / output shard)
NHP = 4           # heads per pass
NPASS = H // NHP  # 4
ST = S // 128     # 16 key tiles
DT = D // 128
FT = F // 128
N_CORES = 8
SBC = 256         # x streaming chunk (columns)
NSB = S // SBC    # 8 chunks per pass
TRIW = 896        # triangle mask width: 3*128 + 512

MAX_WAITS = 1     # this walrus build allows one sync-wait per instruction


# ---------------------------------------------------------------------------
# Tile workarounds for the 1-sync-wait-per-instruction walrus limit
# ---------------------------------------------------------------------------
class TileContextFixed(tile.TileContext):
    def _drain_and_barrier(self, tick_clock, wait_clock):
        nc = self.nc
        probe = nc.sync.nop(nofuse=True)
        wait_clock.add_sem_waits(probe.ins, ScopedClock({None: tick_clock.global_clock}))
        si = probe.ins.sync_info
        waits = list(si.on_wait) if si is not None else []
        if len(waits) > MAX_WAITS:
            si.on_wait = waits[:MAX_WAITS]
            rest = waits[MAX_WAITS:]
            for i in range(0, len(rest), MAX_WAITS):
                extra = nc.sync.nop(nofuse=True)
                extra.ins.sync_info = mybir.SyncInfo(
                    on_wait=rest[i:i + MAX_WAITS], on_update=[])
        nc.sync.drain()
        nc.all_engine_barrier()
        assert self.sems is not None
        popped = nc._tile_sem_poison_stack.pop()
        assert popped is self._sem_poison
        nc.clear_and_free_semaphores(list(self.sems.allocated().values()))
        nc.all_engine_barrier()


def legalize_waits(nc, max_waits=MAX_WAITS):
    for fn in nc.m.functions:
        for bb in fn.blocks:
            insts = bb.instructions
            new_insts = []
            changed = False
            for inst in insts:
                si = inst.sync_info
                if si is not None and len(si.on_wait) > max_waits:
                    waits = list(si.on_wait)
                    keep = waits[:max_waits]
                    rest = waits[max_waits:]
                    for i in range(0, len(rest), max_waits):
                        nop = mybir.InstNoOp(
                            name=nc.get_next_instruction_name(),
                            engine=inst.engine, ins=[], outs=[])
                        nop.sync_info = mybir.SyncInfo(
                            on_wait=rest[i:i + max_waits], on_update=[])
                        nc.register_instruction(nop)
                        new_insts.append(nop)
                        changed = True
                    si.on_wait = keep
                new_insts.append(inst)
            if changed:
                insts.clear()
                insts.extend(new_insts)


# ---------------------------------------------------------------------------
# The fused decoder-layer kernel (one core's program; SPMD-uniform)
# ---------------------------------------------------------------------------
def build_fused():
    nc = bass.Bass()
    xt = nc.dram_tensor("xt", [D, S], F32R, kind="ExternalInput")
    xq = nc.dram_tensor("xq", [D, SB], F32R, kind="ExternalInput")
    wqk = nc.dram_tensor("wqk", [2 * D, H * HD], F32R, kind="ExternalInput")
    wvp = nc.dram_tensor("wvp", [128, DT, H * HD], F32R, kind="ExternalInput")
    wob = nc.dram_tensor("wob", [128, H, D], BF16, kind="ExternalInput")
    # scal: packed per-core scalars: [0:8]=qkwg (qkw*gate per sb-pair),
    # [8:24]=rvz (rv*gate per key tile), [24]=unused,
    # [25:41]=per-tile softmax exp bias (0 useful, -100 excluded -> pr==0)
    scal = nc.dram_tensor("scal", [128, 41], F32, kind="ExternalInput")
    mtri = nc.dram_tensor("mtri", [128, TRIW], BF16, kind="ExternalInput")
    wi01 = nc.dram_tensor("wi01", [128, 2, FT, D], BF16, kind="ExternalInput")
    wog = nc.dram_tensor("wog", [128, DT, F], BF16, kind="ExternalInput")
    out = nc.dram_tensor("out", [D, SB], F32, kind="ExternalOutput")

    xt_p = xt.rearrange("(dt p) s -> p dt s", p=128)
    xq_p = xq.rearrange("(dt p) s -> p dt s", p=128)
    wqk_p = wqk.rearrange("(t dt p) f -> p t dt f", p=128, t=2)
    out_p = out.rearrange("(dt p) q -> p dt q", p=128)

    with TileContextFixed(nc) as tc:
      with ExitStack() as top:
        consts = top.enter_context(tc.tile_pool(name="consts", bufs=1))
        eps_sb = consts.tile([128, 1], F32, name="eps_sb")
        nc.vector.memset(eps_sb, EPS)
        id_f = consts.tile([128, 128], F32, name="id_f")
        make_identity(nc, id_f)
        ones_b = consts.tile([128, 128], BF16, name="ones_b")
        nc.vector.memset(ones_b, 1.0)
        ones_f = consts.tile([128, 1], F32, name="ones_f")
        nc.vector.memset(ones_f, 1.0)
        mask_sb = consts.tile([128, TRIW], BF16, name="mask_sb")
        nc.sync.dma_start(out=mask_sb, in_=mtri[:, :])
        scal_sb = consts.tile([128, 41], F32, name="scal_sb")
        nc.sync.dma_start(out=scal_sb, in_=scal[:, :])

        persist = top.enter_context(tc.tile_pool(name="persist", bufs=1))
        # all 16 heads' attention output, transposed: [hd, head, q], bf16
        attnT = persist.tile([128, H, SB], BF16, tag="attnT", name="attnT")

        # ================= attention passes (4 heads each) =================
        for g in range(NPASS):
            gsl = slice(g * NHP * HD, (g + 1) * NHP * HD)
            with ExitStack() as ph:
                wpool = ph.enter_context(tc.tile_pool(name=f"w{g}", bufs=1))
                wk_sb = wpool.tile([128, DT, NHP * HD], F32R, name="wk_sb")
                nc.gpsimd.dma_start(out=wk_sb, in_=wqk_p[:, 1, :, gsl])

                hpool = ph.enter_context(tc.tile_pool(name=f"hd{g}", bufs=1))
                QT = hpool.tile([128, NHP, SB], F32R, tag="QT", name="QT")

                xpool = ph.enter_context(tc.tile_pool(name=f"xc{g}", bufs=3))
                xqueue = []

                def load_x(sb_):
                    xcol = xpool.tile([128, DT, SBC], F32R, tag="x", name="xcol")
                    nc.sync.dma_start(
                        out=xcol, in_=xt_p[:, :, sb_ * SBC:(sb_ + 1) * SBC])
                    xqueue.append(xcol)

                load_x(0)
                load_x(1)

                # ---- Q projection + per-head rmsnorm (own 512 queries =
                # rotated columns 0..511 = xcol chunks 0 and 1) ----
                with ExitStack() as qph:
                    wqpool = qph.enter_context(tc.tile_pool(name=f"wq{g}", bufs=1))
                    wq_sb = wqpool.tile([128, DT, NHP * HD], F32R, name="wq_sb")
                    nc.gpsimd.dma_start(out=wq_sb, in_=wqk_p[:, 0, :, gsl])
                    qsc = qph.enter_context(tc.tile_pool(name=f"qs{g}", bufs=2))
                    qtmp = qph.enter_context(tc.tile_pool(name=f"qt{g}", bufs=2))
                    psq = qph.enter_context(
                        tc.tile_pool(name=f"pq{g}", bufs=2, space="PSUM"))
                    pst = qph.enter_context(
                        tc.tile_pool(name=f"pt{g}", bufs=2, space="PSUM"))
                    for qs in range(4):
                        xsrc = xqueue[qs // 2]
                        xsl = slice((qs % 2) * 128, (qs % 2) * 128 + 128)
                        q_ps = psq.tile([128, 512], F32, tag="q", name="q_ps")
                        for d in range(DT):
                            nc.tensor.matmul(
                                q_ps, xsrc[:, d, xsl], wq_sb[:, d, :],
                                start=(d == 0), stop=(d == DT - 1))
                        sq = qsc.tile([128, HD], F32, tag="sq", name="sq")
                        ssq = qsc.tile([128, 4], F32, tag="ssq", name="ssq")
                        for h in range(NHP):
                            sl = slice(h * HD, (h + 1) * HD)
                            nc.scalar.activation(out=sq, in_=q_ps[:, sl],
                                                 func=AF.Square,
                                                 accum_out=ssq[:, h:h + 1])
                        rq = qsc.tile([128, 4], F32, tag="rq", name="rq")
                        nc.scalar.activation(out=rq, in_=ssq, func=AF.Sqrt,
                                             scale=1.0 / HD, bias=eps_sb)
                        nc.vector.reciprocal(rq, rq)
                        qh = qtmp.tile([128, 512], F32, tag="qh", name="qh")
                        for h in range(NHP):
                            sl = slice(h * HD, (h + 1) * HD)
                            nc.vector.tensor_scalar_mul(
                                qh[:, sl], q_ps[:, sl], rq[:, h:h + 1])
                        qt_ps = pst.tile([128, 512], F32, tag="qt",
                                         name="qt_ps")
                        for h in range(NHP):
                            sl = slice(h * HD, (h + 1) * HD)
                            nc.tensor.transpose(qt_ps[:, sl], qh[:, sl], id_f)
                        nc.vector.tensor_copy(
                            QT[:, :, qs * 128:(qs + 1) * 128], qt_ps)

                # ---- wv load deferred to here (SBUF headroom) ----
                wv_sb = wpool.tile([128, DT, NHP * HD], F32R, name="wv_sb")
                nc.gpsimd.dma_start(out=wv_sb, in_=wvp[:, :, gsl])

                kvdata = ph.enter_context(tc.tile_pool(name=f"kv{g}", bufs=1))
                KT = kvdata.tile([128, NHP, S], F32R, tag="KT", name="KT")
                VA = kvdata.tile([128, ST, NHP, 132], BF16, tag="VA", name="VA")
                rkb = kvdata.tile([128, ST, NHP], F32, tag="rkb", name="rkb")

                # ---- K^T (direct) + V over full S ----
                with ExitStack() as kph:
                    ktmp = kph.enter_context(tc.tile_pool(name=f"kt{g}", bufs=3))
                    psk = kph.enter_context(
                        tc.tile_pool(name=f"pk{g}", bufs=3, space="PSUM"))
                    psv = kph.enter_context(
                        tc.tile_pool(name=f"pv{g}", bufs=2, space="PSUM"))
                    prk = kph.enter_context(
                        tc.tile_pool(name=f"prk{g}", bufs=1, space="PSUM"))
                    rk_ps = prk.tile([128, 64], F32, tag="rk", name="rk_ps")

                    for sb_ in range(NSB):
                        if sb_ + 2 < NSB:
                            load_x(sb_ + 2)
                        xcol = xqueue.pop(0)
                        # K^T for 4 heads: wk chunk stationary, x moving
                        for h in range(NHP):
                            hsl = slice(h * HD, (h + 1) * HD)
                            kt_ps = psk.tile([128, SBC], F32, tag="k", name="kt_ps")
                            for d in range(DT):
                                nc.tensor.matmul(
                                    kt_ps, wk_sb[:, d, hsl], xcol[:, d, :],
                                    start=(d == 0), stop=(d == DT - 1))
                            ksq = ktmp.tile([128, SBC], F32, tag="ksq", name="ksq")
                            nc.scalar.activation(out=ksq, in_=kt_ps,
                                                 func=AF.Square)
                            # zero-gated copy: scale = qkw * gate(sb-pair)
                            nc.vector.tensor_scalar_mul(
                                KT[:, h, sb_ * SBC:(sb_ + 1) * SBC], kt_ps,
                                scal_sb[:, sb_:sb_ + 1])
                            # rk^2 columns via N=1 ones-matmuls
                            for cc in range(2):
                                nc.tensor.matmul(
                                    rk_ps[:, h * 16 + sb_ * 2 + cc:
                                          h * 16 + sb_ * 2 + cc + 1],
                                    ksq[:, cc * 128:(cc + 1) * 128], ones_f,
                                    start=True, stop=True)
                        # V: x chunk stationary, wv moving
                        for sc in range(2):
                            st = sb_ * 2 + sc
                            ssl = slice(sc * 128, (sc + 1) * 128)
                            v_ps = psv.tile([128, 512], F32, tag="v", name="v_ps")
                            for d in range(DT):
                                nc.tensor.matmul(
                                    v_ps, xcol[:, d, ssl], wv_sb[:, d, :],
                                    start=(d == 0), stop=(d == DT - 1))
                            nc.scalar.activation(
                                out=VA[:, st, :, 0:128],
                                in_=v_ps, func=AF.Copy,
                                scale=scal_sb[:, 8 + st:9 + st])
                    # per-head k-rmsnorm scales
                    for h in range(NHP):
                        nc.scalar.activation(
                            out=rkb[:, :, h], in_=rk_ps[:, h * 16:(h + 1) * 16],
                            func=AF.Sqrt, scale=1.0 / HD, bias=eps_sb)
                        nc.vector.reciprocal(rkb[:, :, h], rkb[:, :, h])

                # ---- scores + transposed AV per head ----
                with ExitStack() as sph:
                    ppool = sph.enter_context(tc.tile_pool(name=f"pr{g}", bufs=4))
                    fpool = sph.enter_context(tc.tile_pool(name=f"fn{g}", bufs=2))
                    pslg = sph.enter_context(
                        tc.tile_pool(name=f"pl{g}", bufs=3, space="PSUM"))
                    psat = sph.enter_context(
                        tc.tile_pool(name=f"pa{g}", bufs=2, space="PSUM"))
                    psdn = sph.enter_context(
                        tc.tile_pool(name=f"pd{g}", bufs=2, space="PSUM"))
                    for h in range(NHP):
                        hh = g * NHP + h
                        atT_ps = psat.tile([128, SB], F32, tag="atT", name="atT")
                        den_ps = psdn.tile([128, SB], F32, tag="den", name="den")
                        for r in range(ST):
                            lg = pslg.tile([128, SB], F32, tag="lg", name="lg")
                            nc.tensor.matmul(
                                lg, KT[:, h, r * 128:(r + 1) * 128], QT[:, h, :],
                                start=True, stop=True)
                            pr = ppool.tile([128, SB], BF16, tag="pr", name="pr")
                            nc.scalar.activation(out=pr, in_=lg, func=AF.Exp,
                                                 scale=rkb[:, r, h:h + 1],
                                                 bias=scal_sb[:, 25 + r:26 + r])
                            if r <= 3:
                                moff = (3 - r) * 128
                                nc.vector.tensor_tensor(
                                    out=pr, in0=pr,
                                    in1=mask_sb[:, moff:moff + SB],
                                    op=mybir.AluOpType.mult)
                            nc.tensor.matmul(
                                atT_ps, VA[:, r, h, 0:128], pr,
                                start=(r == 0), stop=(r == ST - 1))
                            nc.tensor.matmul(
                                den_ps, ones_b, pr,
                                start=(r == 0), stop=(r == ST - 1))
                        denr = fpool.tile([128, SB], F32, tag="denr", name="denr")
                        nc.vector.reciprocal(denr, den_ps)
                        nc.vector.tensor_tensor(
                            out=attnT[:, hh, :], in0=atT_ps, in1=denr,
                            op=mybir.AluOpType.mult)

        # ========== output projection + residual + MLP rmsnorm ==========
        persist2 = top.enter_context(tc.tile_pool(name="persist2", bufs=1))
        interT = persist2.tile([128, DT, SB], F32, tag="interT", name="interT")
        hT = persist2.tile([128, DT, SB], BF16, tag="hT", name="hT")
        with ExitStack() as ph:
            wopool = ph.enter_context(tc.tile_pool(name="wop", bufs=1))
            wo_sb = wopool.tile([128, H, D], BF16, name="wo_sb")
            nc.gpsimd.dma_start(out=wo_sb, in_=wob[:, :, :])
            xq_sb = wopool.tile([128, DT, SB], F32R, name="xq_sb2")
            nc.gpsimd.dma_start(out=xq_sb, in_=xq_p[:, :, :])
            sqpool = ph.enter_context(tc.tile_pool(name="sqp", bufs=2))
            pso = ph.enter_context(tc.tile_pool(name="pso", bufs=2, space="PSUM"))
            pss = ph.enter_context(tc.tile_pool(name="pss", bufs=1, space="PSUM"))
            ss_ps = pss.tile([128, SB], F32, tag="ss", name="ss_ps")
            for dt in range(DT):
                o_ps = pso.tile([128, SB], F32, tag="o", name="o_ps")
                for h in range(H):
                    nc.tensor.matmul(
                        o_ps, wo_sb[:, h, dt * 128:(dt + 1) * 128],
                        attnT[:, h, :], start=(h == 0), stop=(h == H - 1))
                nc.vector.tensor_tensor(out=interT[:, dt, :], in0=o_ps,
                                        in1=xq_sb[:, dt, :],
                                        op=mybir.AluOpType.add)
                sqi = sqpool.tile([128, SB], BF16, tag="sqi", name="sqi")
                nc.vector.tensor_tensor(out=sqi, in0=interT[:, dt, :],
                                        in1=interT[:, dt, :],
                                        op=mybir.AluOpType.mult)
                nc.tensor.matmul(ss_ps, ones_b, sqi,
                                 start=(dt == 0), stop=(dt == DT - 1))
            rr = wopool.tile([128, SB], F32, name="rr")
            nc.scalar.activation(out=rr, in_=ss_ps, func=AF.Sqrt,
                                 scale=1.0 / D, bias=eps_sb)
            nc.vector.reciprocal(rr, rr)
            for dt in range(DT):
                nc.vector.tensor_tensor(out=hT[:, dt, :], in0=interT[:, dt, :],
                                        in1=rr, op=mybir.AluOpType.mult)

        # ================= gated MLP on the [*, 512] slice =================
        with ExitStack() as ph:
            gpool = ph.enter_context(tc.tile_pool(name="gp", bufs=1))
            g_sb = gpool.tile([128, FT, SB], BF16, tag="g", name="g_sb")
            wpool = ph.enter_context(tc.tile_pool(name="wmlp", bufs=3))
            tpool = ph.enter_context(tc.tile_pool(name="tmlp", bufs=4))
            psab = ph.enter_context(tc.tile_pool(name="psab", bufs=2, space="PSUM"))

            wqueue = []

            def load_w(ft):
                w0c = wpool.tile([128, DT, 128], BF16, tag="w0", name="w0c")
                w1c = wpool.tile([128, DT, 128], BF16, tag="w1", name="w1c")
                nc.gpsimd.dma_start(out=w0c, in_=wi01[:, 0, ft, :])
                nc.gpsimd.dma_start(out=w1c, in_=wi01[:, 1, ft, :])
                wqueue.append((w0c, w1c))

            load_w(0)
            load_w(1)
            for ft in range(FT):
                if ft + 2 < FT:
                    load_w(ft + 2)
                w0c, w1c = wqueue.pop(0)
                a_ps = psab.tile([128, SB], F32, tag="a", name="a_ps")
                for d in range(DT):
                    nc.tensor.matmul(a_ps, w0c[:, d, :], hT[:, d, :],
                                     start=(d == 0), stop=(d == DT - 1))
                b_ps = psab.tile([128, SB], F32, tag="b", name="b_ps")
                for d in range(DT):
                    nc.tensor.matmul(b_ps, w1c[:, d, :], hT[:, d, :],
                                     start=(d == 0), stop=(d == DT - 1))
                ga = tpool.tile([128, SB], BF16, tag="ga", name="ga")
                nc.scalar.activation(out=ga, in_=a_ps, func=AF.Silu)
                gb = tpool.tile([128, SB], BF16, tag="gb", name="gb")
                nc.vector.tensor_copy(gb, b_ps)
                nc.vector.tensor_tensor(out=g_sb[:, ft, :], in0=ga, in1=gb,
                                        op=mybir.AluOpType.mult)

            # ---- second MLP matmul + final residual, streamed per d tile ----
            w2pool = ph.enter_context(tc.tile_pool(name="w2p", bufs=2))
            opool = ph.enter_context(tc.tile_pool(name="op", bufs=3))
            pso2 = ph.enter_context(tc.tile_pool(name="pso2", bufs=2, space="PSUM"))

            w2queue = []

            def load_w2(dt):
                wc = w2pool.tile([128, F], BF16, tag="w2", name="w2c")
                nc.gpsimd.dma_start(out=wc, in_=wog[:, dt, :])
                w2queue.append(wc)

            load_w2(0)
            load_w2(1)
            for dt in range(DT):
                if dt + 2 < DT:
                    load_w2(dt + 2)
                wc = w2queue.pop(0)
                o_ps = pso2.tile([128, SB], F32, tag="o2", name="o2_ps")
                for ft in range(FT):
                    nc.tensor.matmul(o_ps, wc[:, ft * 128:(ft + 1) * 128],
                                     g_sb[:, ft, :],
                                     start=(ft == 0), stop=(ft == FT - 1))
                fin = opool.tile([128, SB], F32, tag="fin", name="fin")
                nc.vector.tensor_tensor(out=fin, in0=o_ps, in1=interT[:, dt, :],
                                        op=mybir.AluOpType.add)
                nc.sync.dma_start(out=out_p[:, dt, :], in_=fin)
    legalize_waits(nc)
    return nc


# ---------------------------------------------------------------------------
# Persistent-jit SPMD runner (zeros folded into the jit body: 1 dispatch/call)
# ---------------------------------------------------------------------------
class SpmdRunner:
    def __init__(self, nc, n_cores=N_CORES):
        install_neuronx_cc_hook()
        self.nc = nc
        self.n_cores = n_cores
        partition_name = nc.partition_id_tensor.name if nc.partition_id_tensor else None
        in_names, out_names, out_avals = [], [], []
        for alloc in nc.m.functions[0].allocations:
            if not isinstance(alloc, mybir.MemoryLocationSet):
                continue
            name = alloc.memorylocations[0].name
            if alloc.kind == "ExternalInput":
                if name != partition_name:
                    in_names.append(name)
            elif alloc.kind == "ExternalOutput":
                shape = tuple(alloc.tensor_shape)
                dtype = mybir.dt.np(alloc.dtype)
                out_avals.append(jax.core.ShapedArray(shape, dtype))
                out_names.append(name)
        self.in_names, self.out_names = in_names, out_names
        self.out_avals = out_avals
        n_params = len(in_names)
        self.n_params = n_params
        all_in_names = list(in_names) + list(out_names)
        if partition_name is not None:
            all_in_names.append(partition_name)

        def _body(*args):
            operands = list(args)
            if partition_name is not None:
                operands.append(partition_id_tensor())
            outs = _bass_exec_p.bind(
                *operands,
                out_avals=tuple(out_avals),
                in_names=tuple(all_in_names),
                out_names=tuple(out_names),
                lowering_input_output_aliases=(),
                sim_require_finite=True,
                sim_require_nnan=True,
                nc=nc,
            )
            return tuple(outs)

        devices = jax.devices()[:n_cores]
        assert len(devices) >= n_cores, f"need {n_cores} neuron cores"
        self.mesh = Mesh(np.asarray(devices[:n_cores]), ("core",))
        n_outs = len(out_names)
        in_specs = (PartitionSpec("core"),) * (n_params + n_outs)
        out_specs = (PartitionSpec("core"),) * n_outs
        self._fn = jax.jit(
            shard_map(_body, mesh=self.mesh, in_specs=in_specs,
                      out_specs=out_specs, check_rep=False),
            keep_unused=True)
        sh = NamedSharding(self.mesh, PartitionSpec("core"))
        self._zeros = [
            jax.device_put(
                np.zeros((n_cores * a.shape[0], *a.shape[1:]), a.dtype), sh)
            for a in out_avals
        ]

    def concat_inputs(self, in_maps):
        per_core = [[np.asarray(m[name]) for name in self.in_names] for m in in_maps]
        return [np.concatenate([per_core[c][i] for c in range(self.n_cores)], axis=0)
                for i in range(self.n_params)]

    def device_put_inputs(self, concat_in):
        sh = NamedSharding(self.mesh, PartitionSpec("core"))
        return [jax.device_put(a, sh) for a in concat_in]

    def __call__(self, concat_in):
        return self._fn(*(list(concat_in) + self._zeros))

    def split_outputs(self, out_arrs):
        return [
            {name: np.asarray(out_arrs[i]).reshape(
                self.n_cores, *self.out_avals[i].shape)[c]
             for i, name in enumerate(self.out_names)}
            for c in range(self.n_cores)
        ]

    def run(self, in_maps):
        return self.split_outputs(self(self.device_put_inputs(
            self.concat_inputs(in_maps))))


_RUNNERS = {}


def _get_runner():
    if "fused" not in _RUNNERS:
        _RUNNERS["fused"] = SpmdRunner(build_fused())
    return _RUNNERS["fused"]


# ---------------------------------------------------------------------------
# Host-side prep + the public kernel() entry point
# ---------------------------------------------------------------------------
def _fused_in_maps(x, ln1_scale, wq, wk, wv, qln_scale, kln_scale, wo, ln2_scale,
                   wi0, wi1, wout):
    wq_f = (wq * ln1_scale[:, None, None]).reshape(D, H * HD).astype(np.float32)
    wk_f = (wk * ln1_scale[:, None, None]).reshape(D, H * HD).astype(np.float32)
    wqk_r = np.ascontiguousarray(np.concatenate([wq_f, wk_f], axis=0))
    wv_f = (wv * ln1_scale[:, None, None]).reshape(D, H * HD).astype(np.float32)
    wvp_r = np.ascontiguousarray(wv_f.reshape(DT, 128, H * HD).transpose(1, 0, 2))
    wob_r = np.ascontiguousarray(
        wo.transpose(1, 0, 2).astype(bf16))            # [HD=128, H, D]
    qkw = (qln_scale * kln_scale).astype(np.float32)          # [HD] per-hd
    wi0_f = (wi0 * ln2_scale[:, None]).astype(bf16)
    wi1_f = (wi1 * ln2_scale[:, None]).astype(bf16)
    wi0_r = wi0_f.reshape(DT, 128, FT, 128).transpose(1, 2, 0, 3).reshape(128, FT, D)
    wi1_r = wi1_f.reshape(DT, 128, FT, 128).transpose(1, 2, 0, 3).reshape(128, FT, D)
    wi01_r = np.ascontiguousarray(np.stack([wi0_r, wi1_r], axis=1))
    wog_r = np.ascontiguousarray(
        wout.astype(bf16).reshape(FT, 128, DT, 128).transpose(1, 2, 0, 3)
        .reshape(128, DT, F))

    # shared triangle mask: mtri[p, m] = 1 iff m >= p + 384
    m = np.arange(TRIW)[None, :]
    p = np.arange(128)[:, None]
    mtri = (m >= p + 384).astype(np.float32).astype(bf16)

    # host rv: 1/rms(x) per position
    rv = 1.0 / np.sqrt((x.astype(np.float32) ** 2).mean(-1) + EPS)   # [B, S]

    in_maps = []
    xts = [np.ascontiguousarray(x[b].T) for b in range(B)]       # [D, S]
    for c in range(N_CORES):
        b, j = c // 4, c % 4
        xtb = xts[b]
        xt_rot = np.ascontiguousarray(
            np.roll(xtb, -SB * j, axis=1)).astype(np.float32)
        xq_c = np.ascontiguousarray(xtb[:, j * SB:(j + 1) * SB]).astype(np.float32)
        # gate(j, r) = 1 iff r<=3 or r>=16-4j  (rotated tile r useful)
        gate = np.array([1.0 if (r <= 3 or r >= 16 - 4 * j) else 0.0
                         for r in range(ST)], np.float32)
        n_gated = float((gate == 0.0).sum())
        # rvz[p, r] = gate[r] * rv at rotated position
        rvz_c = np.empty((128, ST), np.float32)
        for r in range(ST):
            a = (4 * j + r) % ST
            rvz_c[:, r] = gate[r] * rv[b, a * 128:(a + 1) * 128]
        # qkwg[p, sb] = qkw[p] * gate[2*sb]  (gate constant within sb-pair)
        qkwg_c = qkw[:, None] * gate[::2][None, :]               # [128, 8]
        scal_c = np.zeros((128, 41), np.float32)
        scal_c[:, 0:8] = qkwg_c
        scal_c[:, 8:24] = rvz_c
        scal_c[:, 25:41] = -100.0 * (1.0 - gate)[None, :]
        in_maps.append({
            "xt": xt_rot,
            "xq": xq_c,
            "wqk": wqk_r,
            "wvp": wvp_r,
            "wob": wob_r,
            "scal": np.ascontiguousarray(scal_c),
            "mtri": np.ascontiguousarray(mtri),
            "wi01": wi01_r,
            "wog": wog_r,
        })
    return in_maps


def kernel(x, ln1_scale, wq, wk, wv, qln_scale, kln_scale, wo, ln2_scale,
           wi0, wi1, wout):
    x = np.asarray(x, np.float32)
    ln1_scale = np.asarray(ln1_scale, np.float32)
    wq = np.asarray(wq, np.float32)
    wk = np.asarray(wk, np.float32)
    wv = np.asarray(wv, np.float32)
    qln_scale = np.asarray(qln_scale, np.float32)
    kln_scale = np.asarray(kln_scale, np.float32)
    wo = np.asarray(wo, np.float32)
    ln2_scale = np.asarray(ln2_scale, np.float32)
    wi0 = np.asarray(wi0, np.float32)
    wi1 = np.asarray(wi1, np.float32)
    wout = np.asarray(wout, np.float32)

    runner = _get_runner()
    maps = _fused_in_maps(x, ln1_scale, wq, wk, wv, qln_scale, kln_scale, wo,
                          ln2_scale, wi0, wi1, wout)
    res = runner.run(maps)
    out = np.empty((B, S, D), np.float32)
    for c in range(N_CORES):
        b, j = c // 4, c % 4
        out[b, j * SB:(j + 1) * SB, :] = res[c]["out"].T
    return out


# revision 23
# speedup vs baseline: 1.2058x; 1.2058x over previous
"""Trainium2 Bass kernel for nn_DecoderLayer_70205535421363 (v2).

Decoder layer (pre-LN, T5-style RMSNorm, QK-norm attention + gated-silu MLP)
B=2, S=2048, D=2048, H=16, HD=128, F=8192, fp32 in/out.

8 cores = 2 batches x 4 query blocks of 512, single-launch, no collectives.

v2 design vs the original fused kernel:
  - Per-core COLUMN ROTATION: core (b, j) stores x[b].T with columns rotated
    left by 512*j, so its own query block is always columns 0..511 and the
    causal structure becomes uniform across cores: rotated key tiles r<=3 get
    a shared triangular mask; later tiles are either fully allowed (wrapped
    past keys) or fully excluded (future keys). Exclusion is done with DATA,
    not instructions: the host zeroes V (rvz) and K (qkwg scale) for excluded
    tiles, so excluded keys contribute exp(0)=1 to the softmax denominator,
    which the host-corrected bias (ngc) subtracts exactly.
  - Direct-KT: K^T[hd, s] is computed directly (wk chunk stationary, x
    moving), eliminating all K transposes and copies. Per-head k-rmsnorm
    scales come from N=1 ones-matmuls on a squared copy of K^T.
  - Transposed AV: att^T[hd, q] = sum_r V_chunk^T @ pr accumulated in PSUM
    (V chunk stationary), plus a broadcast-denominator matmul with an all-ones
    stationary.  No per-qs AV matmuls, no attention-output transposes.
  - rv (1/rms of x) is computed on the host and folded into V (rvz).
  - bf16 everywhere in attention except: q/k psums + norms (f32 accumulation),
    softmax logits (f32 psum), residual (f32), MLP psums (f32).
Two half-passes of 8 heads each keep SBUF under budget while streaming x
only twice.
"""
import numpy as np
import ml_dtypes
from contextlib import ExitStack

import jax
import jax.numpy as jnp
from jax.sharding import Mesh, PartitionSpec, NamedSharding
from jax.experimental.shard_map import shard_map

import concourse.bass as bass
import concourse.tile as tile
import concourse.mybir as mybir
from concourse.bass2jax import _bass_exec_p, install_neuronx_cc_hook, partition_id_tensor
from concourse.vector_clock import ScopedClock
from concourse.masks import make_identity

F32 = mybir.dt.float32
F32R = mybir.dt.float32r
BF16 = mybir.dt.bfloat16
AF = mybir.ActivationFunctionType
bf16 = ml_dtypes.bfloat16

B, S, D, H, HD, F = 2, 2048, 2048, 16, 128, 8192
EPS = 1e-6
SB = 512          # seq positions per core (queries / MLP / output shard)
NHP = 4           # heads per pass
NPASS = H // NHP  # 4
ST = S // 128     # 16 key tiles
DT = D // 128
FT = F // 128
N_CORES = 8
SBC = 256         # x streaming chunk (columns)
NSB = S // SBC    # 8 chunks per pass
TRIW = 896        # triangle mask width: 3*128 + 512

MAX_WAITS = 1     # this walrus build allows one sync-wait per instruction


# ---------------------------------------------------------------------------
# Tile workarounds for the 1-sync-wait-per-instruction walrus limit
# ---------------------------------------------------------------------------
class TileContextFixed(tile.TileContext):
    def _drain_and_barrier(self, tick_clock, wait_clock):
        nc = self.nc
        probe = nc.sync.nop(nofuse=True)
        wait_clock.add_sem_waits(probe.ins, ScopedClock({None: tick_clock.global_clock}))
        si = probe.ins.sync_info
        waits = list(si.on_wait) if si is not None else []
        if len(waits) > MAX_WAITS:
            si.on_wait = waits[:MAX_WAITS]
            rest = waits[MAX_WAITS:]
            for i in range(0, len(rest), MAX_WAITS):
                extra = nc.sync.nop(nofuse=True)
                extra.ins.sync_info = mybir.SyncInfo(
                    on_wait=rest[i:i + MAX_WAITS], on_update=[])
        nc.sync.drain()
        nc.all_engine_barrier()
        assert self.sems is not None
        popped = nc._tile_sem_poison_stack.pop()
        assert popped is self._sem_poison
        nc.clear_and_free_semaphores(list(self.sems.allocated().values()))
        nc.all_engine_barrier()


def legalize_waits(nc, max_waits=MAX_WAITS):
    for fn in nc.m.functions:
        for bb in fn.blocks:
            insts = bb.instructions
            new_insts = []
            changed = False
            for inst in insts:
                si = inst.sync_info
                if si is not None and len(si.on_wait) > max_waits:
                    waits = list(si.on_wait)
                    keep = waits[:max_waits]
                    rest = waits[max_waits:]
                    for i in range(0, len(rest), max_waits):
                        nop = mybir.InstNoOp(
                            name=nc.get_next_instruction_name(),
                            engine=inst.engine, ins=[], outs=[])
                        nop.sync_info = mybir.SyncInfo(
                            on_wait=rest[i:i + max_waits], on_update=[])
                        nc.register_instruction(nop)
                        new_insts.append(nop)
                        changed = True
                    si.on_wait = keep
                new_insts.append(inst)
            if changed:
                insts.clear()
                insts.extend(new_insts)


# ---------------------------------------------------------------------------
# The fused decoder-layer kernel (one core's program; SPMD-uniform)
# ---------------------------------------------------------------------------
def build_fused():
    nc = bass.Bass()
    # xt: rotated x, pre-chunked p-major: [chunk, 128, DT*SBC] contiguous
    xt = nc.dram_tensor("xt", [NSB, 128, DT * SBC], F32R, kind="ExternalInput")
    xq = nc.dram_tensor("xq", [128, DT * SB], F32R, kind="ExternalInput")
    # wqk: [pass, q/k, 128, DT*NHP*HD] contiguous per slice
    wqk = nc.dram_tensor("wqk", [NPASS, 2, 128, DT * NHP * HD], F32R,
                         kind="ExternalInput")
    wvp = nc.dram_tensor("wvp", [NPASS, 128, DT * NHP * HD], F32R,
                         kind="ExternalInput")
    wob = nc.dram_tensor("wob", [128, H * D], BF16, kind="ExternalInput")
    # scal: packed per-core scalars: [0:8]=qkwg (qkw*gate per sb-pair),
    # [8:24]=rvz (rv*gate per key tile), [24]=unused,
    # [25:41]=per-tile softmax exp bias (0 useful, -100 excluded -> pr==0)
    scal = nc.dram_tensor("scal", [128, 41], F32, kind="ExternalInput")
    mtri = nc.dram_tensor("mtri", [128, TRIW], BF16, kind="ExternalInput")
    wi01 = nc.dram_tensor("wi01", [128, 2, FT, D], BF16, kind="ExternalInput")
    wog = nc.dram_tensor("wog", [128, DT, F], BF16, kind="ExternalInput")
    out = nc.dram_tensor("out", [D, SB], F32, kind="ExternalOutput")

    out_p = out.rearrange("(dt p) q -> p dt q", p=128)

    with TileContextFixed(nc) as tc:
      with ExitStack() as top:
        consts = top.enter_context(tc.tile_pool(name="consts", bufs=1))
        eps_sb = consts.tile([128, 1], F32, name="eps_sb")
        nc.vector.memset(eps_sb, EPS)
        id_f = consts.tile([128, 128], F32, name="id_f")
        make_identity(nc, id_f)
        ones_b = consts.tile([128, 128], BF16, name="ones_b")
        nc.vector.memset(ones_b, 1.0)
        ones_f = consts.tile([128, 1], F32, name="ones_f")
        nc.vector.memset(ones_f, 1.0)
        mask_sb = consts.tile([128, TRIW], BF16, name="mask_sb")
        nc.sync.dma_start(out=mask_sb, in_=mtri[:, :])
        scal_sb = consts.tile([128, 41], F32, name="scal_sb")
        nc.sync.dma_start(out=scal_sb, in_=scal[:, :])

        persist = top.enter_context(tc.tile_pool(name="persist", bufs=1))
        # all 16 heads' attention output, transposed: [hd, head, q], bf16
        attnT = persist.tile([128, H, SB], BF16, tag="attnT", name="attnT")

        attn_stack = top.enter_context(ExitStack())
        xpool = attn_stack.enter_context(tc.tile_pool(name="xc", bufs=2))
        wqpool_t = attn_stack.enter_context(tc.tile_pool(name="wqp", bufs=1))
        wkpool_t = attn_stack.enter_context(tc.tile_pool(name="wkp", bufs=1))
        wvpool_t = attn_stack.enter_context(tc.tile_pool(name="wvp", bufs=1))

        # ================= attention passes (4 heads each) =================
        for g in range(NPASS):
            gsl = slice(g * NHP * HD, (g + 1) * NHP * HD)
            with ExitStack() as ph:
                wq_sb = wqpool_t.tile([128, DT, NHP * HD], F32R, tag="wq",
                                      name="wq_sb")
                nc.gpsimd.dma_start(out=wq_sb, in_=wqk[g, 0])
                wk_sb = wkpool_t.tile([128, DT, NHP * HD], F32R, tag="wk",
                                      name="wk_sb")
                nc.gpsimd.dma_start(out=wk_sb, in_=wqk[g, 1])
                wv_sb = wvpool_t.tile([128, DT, NHP * HD], F32R, tag="wv",
                                      name="wv_sb")
                nc.gpsimd.dma_start(out=wv_sb, in_=wvp[g])

                hpool = ph.enter_context(tc.tile_pool(name=f"hd{g}", bufs=1))
                QT = hpool.tile([128, NHP, SB], F32R, tag="QT", name="QT")

                xqueue = []

                def load_x(sb_):
                    xcol = xpool.tile([128, DT, SBC], F32R, tag="x", name="xcol")
                    nc.sync.dma_start(out=xcol, in_=xt[sb_])
                    xqueue.append(xcol)

                load_x(0)
                load_x(1)

                # ---- Q projection + per-head rmsnorm (own 512 queries =
                # rotated columns 0..511 = xcol chunks 0 and 1) ----
                with ExitStack() as qph:
                    qsc = qph.enter_context(tc.tile_pool(name=f"qs{g}", bufs=2))
                    qtmp = qph.enter_context(tc.tile_pool(name=f"qt{g}", bufs=2))
                    psq = qph.enter_context(
                        tc.tile_pool(name=f"pq{g}", bufs=2, space="PSUM"))
                    pst = qph.enter_context(
                        tc.tile_pool(name=f"pt{g}", bufs=2, space="PSUM"))
                    for qs in range(4):
                        xsrc = xqueue[qs // 2]
                        xsl = slice((qs % 2) * 128, (qs % 2) * 128 + 128)
                        q_ps = psq.tile([128, 512], F32, tag="q", name="q_ps")
                        for d in range(DT):
                            nc.tensor.matmul(
                                q_ps, xsrc[:, d, xsl], wq_sb[:, d, :],
                                start=(d == 0), stop=(d == DT - 1))
                        sq = qsc.tile([128, HD], F32, tag="sq", name="sq")
                        ssq = qsc.tile([128, 4], F32, tag="ssq", name="ssq")
                        for h in range(NHP):
                            sl = slice(h * HD, (h + 1) * HD)
                            nc.scalar.activation(out=sq, in_=q_ps[:, sl],
                                                 func=AF.Square,
                                                 accum_out=ssq[:, h:h + 1])
                        rq = qsc.tile([128, 4], F32, tag="rq", name="rq")
                        nc.scalar.activation(out=rq, in_=ssq, func=AF.Sqrt,
                                             scale=1.0 / HD, bias=eps_sb)
                        nc.vector.reciprocal(rq, rq)
                        qh = qtmp.tile([128, 512], F32, tag="qh", name="qh")
                        for h in range(NHP):
                            sl = slice(h * HD, (h + 1) * HD)
                            nc.vector.tensor_scalar_mul(
                                qh[:, sl], q_ps[:, sl], rq[:, h:h + 1])
                        qt_ps = pst.tile([128, 512], F32, tag="qt",
                                         name="qt_ps")
                        for h in range(NHP):
                            sl = slice(h * HD, (h + 1) * HD)
                            nc.tensor.transpose(qt_ps[:, sl], qh[:, sl], id_f)
                        nc.vector.tensor_copy(
                            QT[:, :, qs * 128:(qs + 1) * 128], qt_ps)

                kvdata = ph.enter_context(tc.tile_pool(name=f"kv{g}", bufs=1))
                KT = kvdata.tile([128, NHP, S], F32R, tag="KT", name="KT")
                VA = kvdata.tile([128, ST, NHP, 128], BF16, tag="VA", name="VA")
                rkb = kvdata.tile([128, ST, NHP], F32, tag="rkb", name="rkb")

                # ---- K^T (direct) + V over full S ----
                with ExitStack() as kph:
                    ktmp = kph.enter_context(tc.tile_pool(name=f"kt{g}", bufs=2))
                    psk = kph.enter_context(
                        tc.tile_pool(name=f"pk{g}", bufs=3, space="PSUM"))
                    psv = kph.enter_context(
                        tc.tile_pool(name=f"pv{g}", bufs=2, space="PSUM"))
                    prk = kph.enter_context(
                        tc.tile_pool(name=f"prk{g}", bufs=1, space="PSUM"))
                    rk_ps = prk.tile([128, 64], F32, tag="rk", name="rk_ps")

                    for sb_ in range(NSB):
                        if sb_ + 2 < NSB:
                            load_x(sb_ + 2)
                        xcol = xqueue.pop(0)
                        # K^T for 4 heads: wk chunk stationary, x moving
                        for h in range(NHP):
                            hsl = slice(h * HD, (h + 1) * HD)
                            kt_ps = psk.tile([128, SBC], F32, tag="k", name="kt_ps")
                            for d in range(DT):
                                nc.tensor.matmul(
                                    kt_ps, wk_sb[:, d, hsl], xcol[:, d, :],
                                    start=(d == 0), stop=(d == DT - 1))
                            ksq = ktmp.tile([128, SBC], F32, tag="ksq", name="ksq")
                            nc.scalar.activation(out=ksq, in_=kt_ps,
                                                 func=AF.Square)
                            # zero-gated copy: scale = qkw * gate(sb-pair)
                            nc.vector.tensor_scalar_mul(
                                KT[:, h, sb_ * SBC:(sb_ + 1) * SBC], kt_ps,
                                scal_sb[:, sb_:sb_ + 1])
                            # rk^2 columns via N=1 ones-matmuls
                            for cc in range(2):
                                nc.tensor.matmul(
                                    rk_ps[:, h * 16 + sb_ * 2 + cc:
                                          h * 16 + sb_ * 2 + cc + 1],
                                    ksq[:, cc * 128:(cc + 1) * 128], ones_f,
                                    start=True, stop=True)
                        # V: x chunk stationary, wv moving
                        for sc in range(2):
                            st = sb_ * 2 + sc
                            ssl = slice(sc * 128, (sc + 1) * 128)
                            v_ps = psv.tile([128, 512], F32, tag="v", name="v_ps")
                            for d in range(DT):
                                nc.tensor.matmul(
                                    v_ps, xcol[:, d, ssl], wv_sb[:, d, :],
                                    start=(d == 0), stop=(d == DT - 1))
                            nc.scalar.activation(
                                out=VA[:, st, :, :],
                                in_=v_ps, func=AF.Copy,
                                scale=scal_sb[:, 8 + st:9 + st])
                    # per-head k-rmsnorm scales
                    for h in range(NHP):
                        nc.scalar.activation(
                            out=rkb[:, :, h], in_=rk_ps[:, h * 16:(h + 1) * 16],
                            func=AF.Sqrt, scale=1.0 / HD, bias=eps_sb)
                        nc.vector.reciprocal(rkb[:, :, h], rkb[:, :, h])

                # ---- scores + transposed AV per head ----
                with ExitStack() as sph:
                    ppool = sph.enter_context(tc.tile_pool(name=f"pr{g}", bufs=2))
                    fpool = sph.enter_context(tc.tile_pool(name=f"fn{g}", bufs=1))
                    pslg = sph.enter_context(
                        tc.tile_pool(name=f"pl{g}", bufs=3, space="PSUM"))
                    psat = sph.enter_context(
                        tc.tile_pool(name=f"pa{g}", bufs=2, space="PSUM"))
                    psdn = sph.enter_context(
                        tc.tile_pool(name=f"pd{g}", bufs=2, space="PSUM"))
                    for h in range(NHP):
                        hh = g * NHP + h
                        atT_ps = psat.tile([128, SB], F32, tag="atT", name="atT")
                        den_ps = psdn.tile([128, SB], F32, tag="den", name="den")
                        for r in range(ST):
                            lg = pslg.tile([128, SB], F32, tag="lg", name="lg")
                            nc.tensor.matmul(
                                lg, KT[:, h, r * 128:(r + 1) * 128], QT[:, h, :],
                                start=True, stop=True)
                            pr = ppool.tile([128, SB], BF16, tag="pr", name="pr")
                            nc.scalar.activation(out=pr, in_=lg, func=AF.Exp,
                                                 scale=rkb[:, r, h:h + 1],
                                                 bias=scal_sb[:, 25 + r:26 + r])
                            if r <= 3:
                                moff = (3 - r) * 128
                                nc.vector.tensor_tensor(
                                    out=pr, in0=pr,
                                    in1=mask_sb[:, moff:moff + SB],
                                    op=mybir.AluOpType.mult)
                            nc.tensor.matmul(
                                atT_ps, VA[:, r, h, :], pr,
                                start=(r == 0), stop=(r == ST - 1))
                            nc.tensor.matmul(
                                den_ps, ones_b, pr,
                                start=(r == 0), stop=(r == ST - 1))
                        denr = fpool.tile([128, SB], F32, tag="denr", name="denr")
                        nc.vector.reciprocal(denr, den_ps)
                        nc.vector.tensor_tensor(
                            out=attnT[:, hh, :], in0=atT_ps, in1=denr,
                            op=mybir.AluOpType.mult)

        attn_stack.close()

        # ========== output projection + residual + MLP rmsnorm ==========
        persist2 = top.enter_context(tc.tile_pool(name="persist2", bufs=1))
        interT = persist2.tile([128, DT, SB], F32, tag="interT", name="interT")
        hT = persist2.tile([128, DT, SB], BF16, tag="hT", name="hT")

        wmlp = top.enter_context(tc.tile_pool(name="wmlp", bufs=3))
        wqueue = []

        def load_w(ft):
            w0c = wmlp.tile([128, DT, 128], BF16, tag="w0", name="w0c")
            w1c = wmlp.tile([128, DT, 128], BF16, tag="w1", name="w1c")
            nc.gpsimd.dma_start(out=w0c, in_=wi01[:, 0, ft, :])
            nc.gpsimd.dma_start(out=w1c, in_=wi01[:, 1, ft, :])
            wqueue.append((w0c, w1c))

        load_w(0)
        load_w(1)

        with ExitStack() as ph:
            wopool = ph.enter_context(tc.tile_pool(name="wop", bufs=1))
            wo_sb = wopool.tile([128, H, D], BF16, name="wo_sb")
            nc.gpsimd.dma_start(out=wo_sb, in_=wob[:, :])
            xq_sb = wopool.tile([128, DT, SB], F32R, name="xq_sb2")
            for dt in range(DT):
                nc.sync.dma_start(out=xq_sb[:, dt, :],
                                  in_=xq[:, dt * SB:(dt + 1) * SB])
            sqpool = ph.enter_context(tc.tile_pool(name="sqp", bufs=2))
            pso = ph.enter_context(tc.tile_pool(name="pso", bufs=2, space="PSUM"))
            pss = ph.enter_context(tc.tile_pool(name="pss", bufs=1, space="PSUM"))
            ss_ps = pss.tile([128, SB], F32, tag="ss", name="ss_ps")
            for dt in range(DT):
                o_ps = pso.tile([128, SB], F32, tag="o", name="o_ps")
                for h in range(H):
                    nc.tensor.matmul(
                        o_ps, wo_sb[:, h, dt * 128:(dt + 1) * 128],
                        attnT[:, h, :], start=(h == 0), stop=(h == H - 1))
                nc.vector.tensor_tensor(out=interT[:, dt, :], in0=o_ps,
                                        in1=xq_sb[:, dt, :],
                                        op=mybir.AluOpType.add)
                sqi = sqpool.tile([128, SB], BF16, tag="sqi", name="sqi")
                nc.vector.tensor_tensor(out=sqi, in0=interT[:, dt, :],
                                        in1=interT[:, dt, :],
                                        op=mybir.AluOpType.mult)
                nc.tensor.matmul(ss_ps, ones_b, sqi,
                                 start=(dt == 0), stop=(dt == DT - 1))
            rr = wopool.tile([128, SB], F32, name="rr")
            nc.scalar.activation(out=rr, in_=ss_ps, func=AF.Sqrt,
                                 scale=1.0 / D, bias=eps_sb)
            nc.vector.reciprocal(rr, rr)
            for dt in range(DT):
                nc.vector.tensor_tensor(out=hT[:, dt, :], in0=interT[:, dt, :],
                                        in1=rr, op=mybir.AluOpType.mult)

        # ================= gated MLP on the [*, 512] slice =================
        with ExitStack() as ph:
            gpool = ph.enter_context(tc.tile_pool(name="gp", bufs=1))
            g_sb = gpool.tile([128, FT, SB], BF16, tag="g", name="g_sb")
            tpool = ph.enter_context(tc.tile_pool(name="tmlp", bufs=4))
            psab = ph.enter_context(tc.tile_pool(name="psab", bufs=2, space="PSUM"))

            for ft in range(FT):
                if ft + 2 < FT:
                    load_w(ft + 2)
                w0c, w1c = wqueue.pop(0)
                a_ps = psab.tile([128, SB], F32, tag="a", name="a_ps")
                for d in range(DT):
                    nc.tensor.matmul(a_ps, w0c[:, d, :], hT[:, d, :],
                                     start=(d == 0), stop=(d == DT - 1))
                b_ps = psab.tile([128, SB], F32, tag="b", name="b_ps")
                for d in range(DT):
                    nc.tensor.matmul(b_ps, w1c[:, d, :], hT[:, d, :],
                                     start=(d == 0), stop=(d == DT - 1))
                ga = tpool.tile([128, SB], BF16, tag="ga", name="ga")
                nc.scalar.activation(out=ga, in_=a_ps, func=AF.Silu)
                gb = tpool.tile([128, SB], BF16, tag="gb", name="gb")
                nc.vector.tensor_copy(gb, b_ps)
                nc.vector.tensor_tensor(out=g_sb[:, ft, :], in0=ga, in1=gb,
                                        op=mybir.AluOpType.mult)

            # ---- second MLP matmul + final residual, streamed per d tile ----
            w2pool = ph.enter_context(tc.tile_pool(name="w2p", bufs=2))
            opool = ph.enter_context(tc.tile_pool(name="op", bufs=3))
            pso2 = ph.enter_context(tc.tile_pool(name="pso2", bufs=2, space="PSUM"))

            w2queue = []

            def load_w2(dt):
                wc = w2pool.tile([128, F], BF16, tag="w2", name="w2c")
                nc.gpsimd.dma_start(out=wc, in_=wog[:, dt, :])
                w2queue.append(wc)

            load_w2(0)
            load_w2(1)
            for dt in range(DT):
                if dt + 2 < DT:
                    load_w2(dt + 2)
                wc = w2queue.pop(0)
                o_ps = pso2.tile([128, SB], F32, tag="o2", name="o2_ps")
                for ft in range(FT):
                    nc.tensor.matmul(o_ps, wc[:, ft * 128:(ft + 1) * 128],
                                     g_sb[:, ft, :],
                                     start=(ft == 0), stop=(ft == FT - 1))
                fin = opool.tile([128, SB], F32, tag="fin", name="fin")
                nc.vector.tensor_tensor(out=fin, in0=o_ps, in1=interT[:, dt, :],
                                        op=mybir.AluOpType.add)
                nc.sync.dma_start(out=out_p[:, dt, :], in_=fin)
    legalize_waits(nc)
    return nc


# ---------------------------------------------------------------------------
# Persistent-jit SPMD runner (zeros folded into the jit body: 1 dispatch/call)
# ---------------------------------------------------------------------------
class SpmdRunner:
    def __init__(self, nc, n_cores=N_CORES):
        install_neuronx_cc_hook()
        self.nc = nc
        self.n_cores = n_cores
        partition_name = nc.partition_id_tensor.name if nc.partition_id_tensor else None
        in_names, out_names, out_avals = [], [], []
        for alloc in nc.m.functions[0].allocations:
            if not isinstance(alloc, mybir.MemoryLocationSet):
                continue
            name = alloc.memorylocations[0].name
            if alloc.kind == "ExternalInput":
                if name != partition_name:
                    in_names.append(name)
            elif alloc.kind == "ExternalOutput":
                shape = tuple(alloc.tensor_shape)
                dtype = mybir.dt.np(alloc.dtype)
                out_avals.append(jax.core.ShapedArray(shape, dtype))
                out_names.append(name)
        self.in_names, self.out_names = in_names, out_names
        self.out_avals = out_avals
        n_params = len(in_names)
        self.n_params = n_params
        all_in_names = list(in_names) + list(out_names)
        if partition_name is not None:
            all_in_names.append(partition_name)

        def _body(*args):
            operands = list(args)
            if partition_name is not None:
                operands.append(partition_id_tensor())
            outs = _bass_exec_p.bind(
                *operands,
                out_avals=tuple(out_avals),
                in_names=tuple(all_in_names),
                out_names=tuple(out_names),
                lowering_input_output_aliases=(),
                sim_require_finite=True,
                sim_require_nnan=True,
                nc=nc,
            )
            return tuple(outs)

        devices = jax.devices()[:n_cores]
        assert len(devices) >= n_cores, f"need {n_cores} neuron cores"
        self.mesh = Mesh(np.asarray(devices[:n_cores]), ("core",))
        n_outs = len(out_names)
        in_specs = (PartitionSpec("core"),) * (n_params + n_outs)
        out_specs = (PartitionSpec("core"),) * n_outs
        self._fn = jax.jit(
            shard_map(_body, mesh=self.mesh, in_specs=in_specs,
                      out_specs=out_specs, check_rep=False),
            keep_unused=True)
        sh = NamedSharding(self.mesh, PartitionSpec("core"))
        self._zeros = [
            jax.device_put(
                np.zeros((n_cores * a.shape[0], *a.shape[1:]), a.dtype), sh)
            for a in out_avals
        ]

    def concat_inputs(self, in_maps):
        per_core = [[np.asarray(m[name]) for name in self.in_names] for m in in_maps]
        return [np.concatenate([per_core[c][i] for c in range(self.n_cores)], axis=0)
                for i in range(self.n_params)]

    def device_put_inputs(self, concat_in):
        sh = NamedSharding(self.mesh, PartitionSpec("core"))
        return [jax.device_put(a, sh) for a in concat_in]

    def __call__(self, concat_in):
        return self._fn(*(list(concat_in) + self._zeros))

    def split_outputs(self, out_arrs):
        return [
            {name: np.asarray(out_arrs[i]).reshape(
                self.n_cores, *self.out_avals[i].shape)[c]
             for i, name in enumerate(self.out_names)}
            for c in range(self.n_cores)
        ]

    def run(self, in_maps):
        return self.split_outputs(self(self.device_put_inputs(
            self.concat_inputs(in_maps))))


_RUNNERS = {}


def _get_runner():
    if "fused" not in _RUNNERS:
        _RUNNERS["fused"] = SpmdRunner(build_fused())
    return _RUNNERS["fused"]


# ---------------------------------------------------------------------------
# Host-side prep + the public kernel() entry point
# ---------------------------------------------------------------------------
def _fused_in_maps(x, ln1_scale, wq, wk, wv, qln_scale, kln_scale, wo, ln2_scale,
                   wi0, wi1, wout):
    def pmajor(w):
        # [D, H*HD] -> [NPASS, 128(p), DT, NHP*HD] contiguous per pass
        r = w.reshape(DT, 128, NPASS, NHP * HD).transpose(2, 1, 0, 3)
        return np.ascontiguousarray(r.reshape(NPASS, 128, DT * NHP * HD))

    wq_f = (wq * ln1_scale[:, None, None]).reshape(D, H * HD).astype(np.float32)
    wk_f = (wk * ln1_scale[:, None, None]).reshape(D, H * HD).astype(np.float32)
    wqk_r = np.ascontiguousarray(
        np.stack([pmajor(wq_f), pmajor(wk_f)], axis=1))
    wv_f = (wv * ln1_scale[:, None, None]).reshape(D, H * HD).astype(np.float32)
    wvp_r = pmajor(wv_f)
    wob_r = np.ascontiguousarray(
        wo.transpose(1, 0, 2).astype(bf16).reshape(128, H * D))
    qkw = (qln_scale * kln_scale).astype(np.float32)          # [HD] per-hd
    wi0_f = (wi0 * ln2_scale[:, None]).astype(bf16)
    wi1_f = (wi1 * ln2_scale[:, None]).astype(bf16)
    wi0_r = wi0_f.reshape(DT, 128, FT, 128).transpose(1, 2, 0, 3).reshape(128, FT, D)
    wi1_r = wi1_f.reshape(DT, 128, FT, 128).transpose(1, 2, 0, 3).reshape(128, FT, D)
    wi01_r = np.ascontiguousarray(np.stack([wi0_r, wi1_r], axis=1))
    wog_r = np.ascontiguousarray(
        wout.astype(bf16).reshape(FT, 128, DT, 128).transpose(1, 2, 0, 3)
        .reshape(128, DT, F))

    # shared triangle mask: mtri[p, m] = 1 iff m >= p + 384
    m = np.arange(TRIW)[None, :]
    p = np.arange(128)[:, None]
    mtri = (m >= p + 384).astype(np.float32).astype(bf16)

    # host rv: 1/rms(x) per position
    rv = 1.0 / np.sqrt((x.astype(np.float32) ** 2).mean(-1) + EPS)   # [B, S]

    in_maps = []
    xts = [np.ascontiguousarray(x[b].T) for b in range(B)]       # [D, S]
    for c in range(N_CORES):
        b, j = c // 4, c % 4
        xtb = xts[b]
        xrot = np.roll(xtb, -SB * j, axis=1).astype(np.float32)   # [D, S]
        # chunked p-major: [NSB, 128, DT*SBC]
        xt_rot = np.ascontiguousarray(
            xrot.reshape(DT, 128, NSB, SBC).transpose(2, 1, 0, 3)
            .reshape(NSB, 128, DT * SBC))
        xq_c = np.ascontiguousarray(
            xtb[:, j * SB:(j + 1) * SB].astype(np.float32)
            .reshape(DT, 128, SB).transpose(1, 0, 2).reshape(128, DT * SB))
        # gate(j, r) = 1 iff r<=3 or r>=16-4j  (rotated tile r useful)
        gate = np.array([1.0 if (r <= 3 or r >= 16 - 4 * j) else 0.0
                         for r in range(ST)], np.float32)
        n_gated = float((gate == 0.0).sum())
        # rvz[p, r] = gate[r] * rv at rotated position
        rvz_c = np.empty((128, ST), np.float32)
        for r in range(ST):
            a = (4 * j + r) % ST
            rvz_c[:, r] = gate[r] * rv[b, a * 128:(a + 1) * 128]
        # qkwg[p, sb] = qkw[p] * gate[2*sb]  (gate constant within sb-pair)
        qkwg_c = qkw[:, None] * gate[::2][None, :]               # [128, 8]
        scal_c = np.zeros((128, 41), np.float32)
        scal_c[:, 0:8] = qkwg_c
        scal_c[:, 8:24] = rvz_c
        scal_c[:, 25:41] = -100.0 * (1.0 - gate)[None, :]
        in_maps.append({
            "xt": xt_rot,
            "xq": xq_c,
            "wqk": wqk_r,
            "wvp": wvp_r,
            "wob": wob_r,
            "scal": np.ascontiguousarray(scal_c),
            "mtri": np.ascontiguousarray(mtri),
            "wi01": wi01_r,
            "wog": wog_r,
        })
    return in_maps


def kernel(x, ln1_scale, wq, wk, wv, qln_scale, kln_scale, wo, ln2_scale,
           wi0, wi1, wout):
    x = np.asarray(x, np.float32)
    ln1_scale = np.asarray(ln1_scale, np.float32)
    wq = np.asarray(wq, np.float32)
    wk = np.asarray(wk, np.float32)
    wv = np.asarray(wv, np.float32)
    qln_scale = np.asarray(qln_scale, np.float32)
    kln_scale = np.asarray(kln_scale, np.float32)
    wo = np.asarray(wo, np.float32)
    ln2_scale = np.asarray(ln2_scale, np.float32)
    wi0 = np.asarray(wi0, np.float32)
    wi1 = np.asarray(wi1, np.float32)
    wout = np.asarray(wout, np.float32)

    runner = _get_runner()
    maps = _fused_in_maps(x, ln1_scale, wq, wk, wv, qln_scale, kln_scale, wo,
                          ln2_scale, wi0, wi1, wout)
    res = runner.run(maps)
    out = np.empty((B, S, D), np.float32)
    for c in range(N_CORES):
        b, j = c // 4, c % 4
        out[b, j * SB:(j + 1) * SB, :] = res[c]["out"].T
    return out
